# revision 64
# baseline (speedup 1.0000x reference)
"""Trainium2 Bass kernel for EnhancedAttentionV2:
sliding-window (256) attention + residual + layernorm, B=1, S=4096, HS=1024,
H=16 heads, D=64. Sequence-parallel across 8 NeuronCores: each core computes
512 query rows; K/V for its 768-key band (128-halo each side) are recomputed
locally from a zero-padded slice of hidden_states, so no collectives are
needed.

Per-core device algorithm:
  Q/K/V projections run in fp8e4 (host-quantized: h x16, W x512) with
  DoubleRow matmuls (256-wide contraction per pass); the PSUM->SBUF copies
  descale by 1/8192 and write bf16 qT/kT (dT-major) and v' (s-major,
  per-head 66-wide slot: 64 v cols | denom col | pad).
  Attention is q-major per (head, query-block qb, band-chunk o in
  {qb,qb+1,qb+2}):
    scoresT unit [128 k, 128 q] = kT-slice.T @ qT-slice   (bf16 matmul)
    8 units pack one [128,1024] PSUM tile; one Exp(0.125 x) activation
    writes bf16 probsT; window masks fold into one bf16 multiply per
    edge-group (tri masks repeated x8).
    ctx[q,.] += probsT-unit.T @ v'-slot  accumulates [128 q, 66] in PSUM
    over the 3 o's; column 64 (ones column, zeroed on pads via a host
    pattern) accumulates the softmax denominator.
  res += ctx * recip(denom) per (head, qb), then layernorm rows -> out.
"""

import os
from contextlib import ExitStack

import numpy as np

import concourse.bass as bass
import concourse.mybir as mybir
import concourse.tile as tile
from concourse import bacc

P = 128
S, HS, H, D = 4096, 1024, 16, 64
N_CORES = 8
SL = S // N_CORES           # 512 local query rows
QB = SL // P                # 4 query blocks
SBAND = SL + 2 * P          # 768 band keys
OB = SBAND // P             # 6 key chunks
KC = HS // P                # 8 contraction chunks
KD = KC // 2                # 4 DoubleRow (256-wide) contraction chunks
VST = 65                    # per-head slot width in v' (64 v | denom col)
EPS = 1e-12
SH = 24.0                   # fp8 scale on hidden_states
SW = 512.0                  # fp8 scale on weights
DS = 1.0 / (SH * SW)        # descale applied on PSUM->SBUF copies
F32 = mybir.dt.float32
BF16 = mybir.dt.bfloat16
F8 = mybir.dt.float8e4
AF = mybir.ActivationFunctionType
ALU = mybir.AluOpType
DRM = mybir.MatmulPerfMode.DoubleRow

# tuning knobs (set by the config sweep; defaults = best known)
CFG = {"proj_first": False, "big_bufs": 3, "ctx_bufs": 1, "qt_eng": "act",
       "ktlo_eng": "act", "defer_stt": True, "qk_bf16": True,
       "v_bf16": False, "v_hilo": False}


def _build(has_b: bool, has_ln: bool, reps: int = 1):
    nc = bacc.Bacc(None, target_bir_lowering=False, debug=False,
                   num_devices=N_CORES)
    QKT = BF16 if CFG["qk_bf16"] else F8
    hT_d = nc.dram_tensor("hT", [HS, SBAND], F8, kind="ExternalInput").ap()
    wq_d = nc.dram_tensor("wqT", [KC, HS, P], QKT, kind="ExternalInput").ap()
    wk_d = nc.dram_tensor("wkT", [KC, HS, P], QKT, kind="ExternalInput").ap()
    if CFG["qk_bf16"]:
        hTb_d = nc.dram_tensor("hTb", [HS, SBAND], BF16,
                               kind="ExternalInput").ap()
    if CFG["v_hilo"]:
        hTl_d = nc.dram_tensor("hTl", [HS, SBAND], F8,
                               kind="ExternalInput").ap()
    VT = BF16 if CFG["v_bf16"] else F8
    wv_d = nc.dram_tensor("wvT", [HS, HS], VT, kind="ExternalInput").ap()
    res_d = nc.dram_tensor("res", [SL, HS], F32, kind="ExternalInput").ap()
    m0_d = nc.dram_tensor("m0x8", [P, 8 * P], BF16, kind="ExternalInput").ap()
    m2_d = nc.dram_tensor("m2x8", [P, 8 * P], BF16, kind="ExternalInput").ap()
    km_d = nc.dram_tensor("kmask", [P, OB * H], BF16, kind="ExternalInput").ap()
    if has_b:
        bq_d = nc.dram_tensor("bq", [HS], F32, kind="ExternalInput").ap()
        bk_d = nc.dram_tensor("bk", [HS], F32, kind="ExternalInput").ap()
        bv_d = nc.dram_tensor("bv", [1, HS], F8, kind="ExternalInput").ap()
    if has_ln:
        gam_d = nc.dram_tensor("gam", [P, HS], F32, kind="ExternalInput").ap()
        bet_d = nc.dram_tensor("bet", [P, HS], F32, kind="ExternalInput").ap()
    # fp16 output: written once (no accumulation), ~0.05% quantization,
    # halves the tail DMA on the serial transfer pipe; host upcasts
    F16 = mybir.dt.float16
    out_d = nc.dram_tensor("out", [SL, HS], F16, kind="ExternalOutput").ap()
    if CFG.get("debug"):
        dbg_qk = nc.dram_tensor("dbg_qk", [P, SL + SBAND], F32,
                                kind="ExternalOutput").ap()
        dbg_v = nc.dram_tensor("dbg_v", [P, OB, H, VST], F32,
                               kind="ExternalOutput").ap()
        dbg_pr = nc.dram_tensor("dbg_pr", [P, 3, 8 * P], F32,
                                kind="ExternalOutput").ap()
        dbg_ctx = nc.dram_tensor("dbg_ctx", [P, 2, 512], F32,
                                 kind="ExternalOutput").ap()

    hT_re = hT_d.rearrange("(c p) j -> c p j", p=P)          # [KC, 128, SBAND]
    wq_re = wq_d.rearrange("m (c p) w -> m p c w", p=P)      # [m][128, KC, 128]
    wk_re = wk_d.rearrange("m (c p) w -> m p c w", p=P)
    wv_re = wv_d.rearrange("(c p) d -> p c d", p=P)          # [128, KC, HS]
    out_re = out_d.rearrange("(q p) d -> p q d", p=P)

    with tile.TileContext(nc) as tc, ExitStack() as ctx:
        const = ctx.enter_context(tc.tile_pool(name="const", bufs=1))
        persist = ctx.enter_context(tc.tile_pool(name="persist", bufs=1))
        probs_p = ctx.enter_context(tc.tile_pool(name="probs", bufs=6))
        qk_sb = ctx.enter_context(tc.tile_pool(name="qksb", bufs=2))
        stats_p = ctx.enter_context(tc.tile_pool(name="stats", bufs=12))
        pp_big = ctx.enter_context(tc.tile_pool(name="ppbig",
                                                bufs=CFG["big_bufs"],
                                                space="PSUM"))
        pp_ctx = ctx.enter_context(tc.tile_pool(name="ppctx",
                                                bufs=CFG["ctx_bufs"],
                                                space="PSUM"))

        m0_t = const.tile([P, 8 * P], BF16)
        m2_t = const.tile([P, 8 * P], BF16)
        eps_t = const.tile([P, 1], F32)
        nc.vector.memset(eps_t[:], EPS)
        if has_b:
            bq_t = const.tile([P, KC], F32)
            nc.gpsimd.dma_start(bq_t[:], bq_d.rearrange("(c p) -> p c", p=P))
            bk_t = const.tile([P, KC], F32)
            nc.gpsimd.dma_start(bk_t[:], bk_d.rearrange("(c p) -> p c", p=P))
            bv_t = const.tile([1, HS], F8)
            nc.gpsimd.dma_start(bv_t[:], bv_d[:])
            ones_t = const.tile([1, P], F8)
            nc.vector.memset(ones_t[:], 1.0)
        if has_ln:
            gam_t = persist.tile([P, HS], F32)
            bet_t = persist.tile([P, HS], F32)
            nc.gpsimd.dma_start(gam_t[:], gam_d[:])
            nc.gpsimd.dma_start(bet_t[:], bet_d[:])

        hT_t = persist.tile([P, KC, SBAND], F8)
        res_t = persist.tile([P, QB, HS], F32)
        v_t = persist.tile([P, OB, H, VST], BF16)
        out_t = persist.tile([P, QB, HS], F16)
        wv_t = persist.tile([P, KC, HS], VT)
        wq_t = persist.tile([P, KC, KC, P], QKT)  # [p, m, c, w]
        wk_t = persist.tile([P, KC, KC, P], QKT)
        if CFG["qk_bf16"]:
            hTb_t = persist.tile([P, KC, SBAND], BF16)
            hTb_re = hTb_d.rearrange("(c p) j -> c p j", p=P)
        if CFG["v_hilo"]:
            hTl_t = persist.tile([P, KC, SBAND], F8)
            hTl_re = hTl_d.rearrange("(c p) j -> c p j", p=P)
        st6_t = persist.tile([P, QB, KC, 6], F32)

        wq_re2 = wq_d.rearrange("m (c p) w -> p m c w", p=P)
        wk_re2 = wk_d.rearrange("m (c p) w -> p m c w", p=P)

        for _rep in range(reps):
            res_re = res_d.rearrange("(q p) d -> p q d", p=P)
            wv_re3 = wv_d.rearrange("(c p) d -> c p d", p=P)
            nc.sync.dma_start(wq_t[:, 0], wq_re[0])
            nc.sync.dma_start(wk_t[:, 0], wk_re[0])
            if CFG["qk_bf16"]:
                for c in range(KC):
                    nc.sync.dma_start(hTb_t[:, c, :], hTb_re[c])
                for c in range(KC):
                    nc.scalar.dma_start(hT_t[:, c, :], hT_re[c])
                if CFG["v_hilo"]:
                    for c in range(KC):
                        nc.gpsimd.dma_start(hTl_t[:, c, :], hTl_re[c])
            else:
                for c in range(KC):
                    nc.sync.dma_start(hT_t[:, c, :], hT_re[c])
            for c in range(KC):
                nc.scalar.dma_start(wv_t[:, c, :], wv_re3[c])
            nc.sync.dma_start(res_t[:, 0, :], res_re[:, 0, :])
            nc.sync.dma_start(res_t[:, 1, :], res_re[:, 1, :])
            for m in range(1, KC):
                nc.sync.dma_start(wq_t[:, m], wq_re[m])
                nc.sync.dma_start(wk_t[:, m], wk_re[m])
            nc.scalar.dma_start(res_t[:, 2, :], res_re[:, 2, :])
            nc.scalar.dma_start(res_t[:, 3, :], res_re[:, 3, :])
            def emit_proj(m):
                # qk_t: [0:512] = qT, [512:1280] = kT (one ACT copy covers
                # qT + kT-lo; kT-hi moves on DVE)
                qk_t = qk_sb.tile([P, SL + SBAND], BF16, tag="qkT")
                qTm = qk_t[:, 0:SL]
                kTm = qk_t[:, SL:SL + SBAND]
                ps1 = pp_big.tile([P, 1024], F32, tag="big")  # Q | K lo
                ps2 = pp_big.tile([P, 1024], F32, tag="big")  # K hi
                if CFG["qk_bf16"]:
                    for c in range(KC):
                        nc.tensor.matmul(ps1[:, 0:512], wq_t[:, m, c, :],
                                         hTb_t[:, c, P:P + SL],
                                         start=(c == 0), stop=(c == KC - 1))
                    for c in range(KC):
                        nc.tensor.matmul(ps1[:, 512:1024], wk_t[:, m, c, :],
                                         hTb_t[:, c, 0:512],
                                         start=(c == 0), stop=(c == KC - 1))
                    for c in range(KC):
                        nc.tensor.matmul(ps2[:, 0:256], wk_t[:, m, c, :],
                                         hTb_t[:, c, 512:768],
                                         start=(c == 0), stop=(c == KC - 1))
                else:
                    for t in range(KD):
                        nc.tensor.matmul(ps1[:, 0:512],
                                         wq_t[:, m, 2 * t:2 * t + 2, :],
                                         hT_t[:, 2 * t:2 * t + 2, P:P + SL],
                                         start=(t == 0), stop=(t == KD - 1),
                                         perf_mode=DRM)
                    for t in range(KD):
                        nc.tensor.matmul(ps1[:, 512:1024],
                                         wk_t[:, m, 2 * t:2 * t + 2, :],
                                         hT_t[:, 2 * t:2 * t + 2, 0:512],
                                         start=(t == 0), stop=(t == KD - 1),
                                         perf_mode=DRM)
                    for t in range(KD):
                        nc.tensor.matmul(ps2[:, 0:256],
                                         wk_t[:, m, 2 * t:2 * t + 2, :],
                                         hT_t[:, 2 * t:2 * t + 2, 512:768],
                                         start=(t == 0), stop=(t == KD - 1),
                                         perf_mode=DRM)
                qkds = 1.0 if CFG["qk_bf16"] else DS
                if has_b:
                    nc.scalar.activation(qTm[:], ps1[:, 0:512], AF.Identity,
                                         bias=bq_t[:, m:m + 1], scale=qkds)
                    nc.scalar.activation(kTm[:, 0:512], ps1[:, 512:1024],
                                         AF.Identity,
                                         bias=bk_t[:, m:m + 1], scale=qkds)
                    nc.scalar.activation(kTm[:, 512:768], ps2[:, 0:256],
                                         AF.Identity,
                                         bias=bk_t[:, m:m + 1], scale=qkds)
                else:
                    nc.scalar.activation(qk_t[:, 0:1024], ps1[:],
                                         AF.Identity, scale=qkds)
                    nc.vector.tensor_scalar_mul(kTm[:, 512:768],
                                                ps2[:, 0:256], qkds)
                if CFG.get("debug") and m == 0:
                    dk = const.tile([P, SL + SBAND], F32, tag="dbgqk")
                    nc.vector.tensor_copy(dk[:], qk_t[:])
                    nc.sync.dma_start(dbg_qk[:], dk[:])
                return qTm, kTm

            def emit_v_projection():
                for sb in range(OB):
                    ps = pp_big.tile([P, 1024], F32, tag="big")
                    for n2 in range(2):
                        if CFG["v_bf16"]:
                            for c in range(KC):
                                nc.tensor.matmul(
                                    ps[:, n2 * 512:(n2 + 1) * 512],
                                    hTb_t[:, c, sb * P:(sb + 1) * P],
                                    wv_t[:, c, n2 * 512:(n2 + 1) * 512],
                                    start=(c == 0),
                                    stop=(c == KC - 1) and not has_b)
                        else:
                            # hi (+ optional lo residual) fp8 DoubleRow
                            # sweeps accumulate at the same PSUM scale
                            srcs = [hT_t] + ([hTl_t] if CFG["v_hilo"] else [])
                            for si, hsrc in enumerate(srcs):
                                for t in range(KD):
                                    nc.tensor.matmul(
                                        ps[:, n2 * 512:(n2 + 1) * 512],
                                        hsrc[:, 2 * t:2 * t + 2,
                                             sb * P:(sb + 1) * P],
                                        wv_t[:, 2 * t:2 * t + 2,
                                             n2 * 512:(n2 + 1) * 512],
                                        start=(si == 0 and t == 0),
                                        stop=(si == len(srcs) - 1
                                              and t == KD - 1) and not has_b,
                                        perf_mode=DRM)
                        if has_b:
                            nc.tensor.matmul(ps[:, n2 * 512:(n2 + 1) * 512],
                                             ones_t[:1, :],
                                             bv_t[:1, n2 * 512:(n2 + 1) * 512],
                                             start=False, stop=True)
                    # raw (un-descaled) bf16 v'; the denominator column is
                    # pre-scaled by SH*SW so ctx/den descales for free
                    eng = nc.scalar if sb % 2 == 0 else nc.vector
                    eng_copy = (nc.scalar.copy if sb % 2 == 0
                                else nc.vector.tensor_copy)
                    eng_copy(v_t[:, sb, :, 0:D],
                             ps[:].rearrange("p (h d) -> p h d", d=D))

            # ---- software-pipelined m loop: scores(m) | proj(m+1) | ctx(m)
            qkQ = [emit_proj(0)]
            nc.gpsimd.dma_start(m0_t[:], m0_d[:])
            nc.gpsimd.dma_start(m2_t[:], m2_d[:])
            # denominator column: SH*SW on valid keys (un-descaled ctx is
            # SH*SW times too big, so the ratio ctx/den comes out exact),
            # 0.0 on pads and attention_mask zeros -- host-staged, one
            # contiguous DMA + one strided on-chip copy into per-head slots.
            kms_t = const.tile([P, OB * H], BF16)
            nc.gpsimd.dma_start(kms_t[:], km_d[:])
            nc.vector.tensor_copy(
                v_t[:, :, :, D:D + 1].rearrange("p o h one -> p o (h one)"),
                kms_t[:].rearrange("p (o h) -> p o h", o=OB))
            unnorm = []
            for m in range(KC):
                qTm, kTm = qkQ.pop(0)
                if m == 0:
                    # V emission here keeps every v' write ahead (in program
                    # order) of the first ctx matmul that reads it.
                    emit_v_projection()
                # un-normalize of the PREVIOUS m at cycle start: its ctx psum
                # is complete, so the DVE runs it while ACT does this m's exps
                unnorm_bn = None
                if unnorm and CFG["defer_stt"] == "front":
                    stt, bn = unnorm.pop()
                    stt()
                    bn()
                if CFG["proj_first"] and m + 1 < KC:
                    qkQ.append(emit_proj(m + 1))
                # 24 (h, qb, o) units of [128 k, 128 q]; 3 packed groups:
                # group 0: o = qb     (M0 upper-edge masks)
                # group 1: o = qb + 2 (M2 lower-edge masks)
                # group 2: o = qb + 1 (interior, unmasked)
                GRP = ((0, m0_t), (2, m2_t), (1, None))
                prs = []
                for g, (oofs, msk) in enumerate(GRP):
                    pssc = pp_big.tile([P, 8 * P], F32, tag="big")
                    for hh in range(2):
                        pb = hh * D
                        for qb in range(QB):
                            u = hh * QB + qb
                            o = qb + oofs
                            nc.tensor.matmul(
                                pssc[:, u * P:(u + 1) * P],
                                kTm[pb:pb + D, o * P:(o + 1) * P],
                                qTm[pb:pb + D, qb * P:(qb + 1) * P],
                                start=True, stop=True)
                    pr = probs_p.tile([P, 8 * P], BF16, tag="probs")
                    nc.scalar.activation(pr[:], pssc[:], AF.Exp, scale=0.125)
                    if msk is not None:
                        nc.vector.tensor_mul(pr[:], pr[:], msk[:])
                    prs.append(pr)
                    if g == 0 and unnorm and CFG["defer_stt"] is True:
                        # previous m's recip+STT right after mask-g0: it
                        # frees the single ctx PSUM before this m's ctx-g0
                        # needs it, while mask-g1 still precedes ctx-g1
                        stt, bn = unnorm.pop()
                        stt()
                        unnorm_bn = bn
                    if CFG.get("debug") and m == 0:
                        dq = const.tile([P, 8 * P], F32, tag=f"dbgpr{g}")
                        nc.vector.tensor_copy(dq[:], pr[:])
                        nc.sync.dma_start(dbg_pr[:, g, :], dq[:])

                if unnorm_bn is not None:
                    unnorm_bn()
                    unnorm_bn = None
                if not CFG["proj_first"] and m + 1 < KC:
                    qkQ.append(emit_proj(m + 1))

                ctxps = pp_ctx.tile([P, 2, 512], F32, tag="ctx")
                for g, (oofs, msk) in enumerate(GRP):
                    pr = prs[g]
                    for hh in range(2):
                        for qb in range(QB):
                            u = hh * QB + qb
                            o = qb + oofs
                            # PSUM start wipes the whole bank's has_written
                            # (bank-granular), so exactly one start per
                            # head-bank; later units clean-write via the
                            # zero-region rule, later groups accumulate.
                            nc.tensor.matmul(
                                ctxps[:, hh, qb * VST:qb * VST + VST],
                                pr[:, u * P:(u + 1) * P],
                                v_t[:, o, 2 * m + hh, :],
                                start=(g == 0 and qb == 0),
                                stop=(g == 2 and qb == QB - 1))

                # ---- un-normalize + residual accumulate (deferred) ----
                def mk_unnorm(m, ctxps):
                    def emit_stt():
                        rc8 = stats_p.tile([P, 2, QB], F32, tag="rc")
                        nc.vector.reciprocal(
                            rc8[:],
                            ctxps[:, :, 0:QB * VST].rearrange(
                                "p h (q v) -> p h q v",
                                v=VST)[:, :, :, D:D + 1]
                            .rearrange("p h q one -> p h (q one)"))
                        for hh in range(2):
                            hcol = (2 * m + hh) * D
                            for qb in range(QB):
                                nc.vector.scalar_tensor_tensor(
                                    res_t[:, qb, hcol:hcol + D],
                                    ctxps[:, hh, qb * VST:qb * VST + D],
                                    rc8[:, hh, qb:qb + 1],
                                    res_t[:, qb, hcol:hcol + D],
                                    op0=ALU.mult, op1=ALU.add)
                    def emit_bn():
                        for qb in range(QB):
                            # incremental layernorm stats for this m's cols
                            nc.vector.bn_stats(
                                st6_t[:, qb, m, :],
                                res_t[:, qb, m * P:(m + 1) * P])
                    return emit_stt, emit_bn
                if CFG.get("debug") and m == 0:
                    dc = const.tile([P, 2, 512], F32, tag="dbgctx")
                    nc.vector.tensor_copy(dc[:], ctxps[:])
                    nc.sync.dma_start(dbg_ctx[:], dc[:])
                    dv = const.tile([P, OB, H, VST], F32, tag="dbgv")
                    nc.vector.tensor_copy(dv[:], v_t[:])
                    nc.sync.dma_start(dbg_v[:], dv[:])
                unnorm.append(mk_unnorm(m, ctxps))
                if not CFG["defer_stt"] or m == KC - 1:
                    while unnorm:
                        stt, bn = unnorm.pop()
                        stt()
                        bn()

            # ---- layernorm over each row of res ----
            for qb in range(QB):
                xq = res_t[:, qb, :]
                mv = stats_p.tile([P, 2], F32, tag="mv")
                nc.vector.bn_aggr(mv[:], st6_t[:, qb, :, :])
                std = stats_p.tile([P, 1], F32, tag="st")
                nc.scalar.activation(std[:], mv[:, 1:2], AF.Sqrt, bias=eps_t[:])
                rstd = stats_p.tile([P, 1], F32, tag="st")
                nc.vector.reciprocal(rstd[:], std[:])
                if qb % 2 == 0:
                    nmr = stats_p.tile([P, 1], F32, tag="st")
                    nc.vector.scalar_tensor_tensor(nmr[:], mv[:, 0:1], -1.0,
                                                   rstd[:], op0=ALU.mult,
                                                   op1=ALU.mult)
                    nc.scalar.activation(out_t[:, qb, :], xq, AF.Identity,
                                         bias=nmr[:], scale=rstd[:])
                else:
                    negmu = stats_p.tile([P, 1], F32, tag="st")
                    nc.vector.tensor_scalar(negmu[:], mv[:, 0:1], -1.0, None,
                                            op0=ALU.mult)
                    nc.vector.tensor_scalar(out_t[:, qb, :], xq, negmu[:],
                                            rstd[:], op0=ALU.add,
                                            op1=ALU.mult)
                if has_ln:
                    nc.vector.tensor_mul(out_t[:, qb, :], out_t[:, qb, :],
                                         gam_t[:])
                    nc.vector.tensor_add(out_t[:, qb, :], out_t[:, qb, :],
                                         bet_t[:])
                oeng = nc.sync if qb % 2 == 0 else nc.scalar
                oeng.dma_start(out_re[:, qb, :], out_t[:, qb, :])

    nc.compile()
    return nc


class _Runner:
    """Reusable jitted SPMD executor for a compiled Bass program.

    Mirrors concourse.bass2jax.run_bass_via_pjrt's multi-core path, but keeps
    the jitted function and device-resident inputs so repeat executions skip
    lowering/compile and host->device staging.
    """

    def __init__(self, nc):
        import jax
        from jax.experimental.shard_map import shard_map
        from jax.sharding import Mesh, NamedSharding, PartitionSpec
        from concourse import bass2jax

        bass2jax.install_neuronx_cc_hook()
        self.nc = nc
        in_names: list[str] = []
        out_names: list[str] = []
        out_avals = []
        zero_outs: list[np.ndarray] = []
        partition_name = (nc.partition_id_tensor.name
                          if nc.partition_id_tensor else None)
        for alloc in nc.m.functions[0].allocations:
            if not isinstance(alloc, mybir.MemoryLocationSet):
                continue
            name = alloc.memorylocations[0].name
            if alloc.kind == "ExternalInput":
                if name != partition_name:
                    in_names.append(name)
            elif alloc.kind == "ExternalOutput":
                shape = tuple(alloc.tensor_shape)
                dtype = mybir.dt.np(alloc.dtype)
                out_names.append(name)
                out_avals.append(jax.core.ShapedArray(shape, dtype))
                zero_outs.append(np.zeros(shape, dtype))
        self.n_params = len(in_names)
        self.in_names = list(in_names)
        self.out_names = out_names
        self.out_avals = out_avals
        self.zero_outs = zero_outs
        all_in_names = in_names + out_names
        if partition_name is not None:
            all_in_names.append(partition_name)

        def _body(*args):
            operands = list(args)
            if partition_name is not None:
                operands.append(bass2jax.partition_id_tensor())
            outs = bass2jax._bass_exec_p.bind(
                *operands,
                out_avals=tuple(out_avals),
                in_names=tuple(all_in_names),
                out_names=tuple(out_names),
                lowering_input_output_aliases=(),
                sim_require_finite=True,
                sim_require_nnan=True,
                nc=nc,
            )
            return tuple(outs)

        devices = jax.devices()[:N_CORES]
        self.mesh = Mesh(np.asarray(devices), ("core",))
        self.sharding = NamedSharding(self.mesh, PartitionSpec("core"))
        n_all = self.n_params + len(out_names)
        self.fn = jax.jit(
            shard_map(_body, mesh=self.mesh,
                      in_specs=(PartitionSpec("core"),) * n_all,
                      out_specs=(PartitionSpec("core"),) * len(out_names),
                      check_rep=False),
            keep_unused=True,
        )

    def stage(self, in_maps):
        import jax
        args = []
        for i, name in enumerate(self.in_names):
            concat = np.concatenate(
                [np.asarray(m[name]) for m in in_maps], axis=0)
            args.append(jax.device_put(concat, self.sharding))
        for z in self.zero_outs:
            zz = np.zeros((N_CORES * z.shape[0], *z.shape[1:]), z.dtype)
            args.append(jax.device_put(zz, self.sharding))
        return args

    def run(self, staged):
        out_arrs = self.fn(*staged)
        return [a.block_until_ready() for a in out_arrs]

    def results(self, out_arrs):
        res = []
        for c in range(N_CORES):
            res.append({
                name: np.asarray(out_arrs[i]).reshape(
                    N_CORES, *self.out_avals[i].shape)[c]
                for i, name in enumerate(self.out_names)
            })
        return res


_CACHE: dict = {}


def _get_runner(has_b: bool, has_ln: bool, reps: int = 1) -> _Runner:
    key = (has_b, has_ln, reps)
    if key not in _CACHE:
        _CACHE[key] = _Runner(_build(has_b, has_ln, reps))
    return _CACHE[key]


def _prep_inputs(hidden_states, attention_mask, Wq, bq, Wk, bk, Wv, bv,
                 ln_gamma, ln_beta):
    import ml_dtypes
    F8NP = ml_dtypes.float8_e4m3
    BFNP = ml_dtypes.bfloat16

    hs = np.asarray(hidden_states, dtype=np.float32)[0]      # [S, HS]
    am = np.asarray(attention_mask, dtype=np.float32)[0]     # [S]
    Wq = np.asarray(Wq, dtype=np.float32)
    Wk = np.asarray(Wk, dtype=np.float32)
    Wv = np.asarray(Wv, dtype=np.float32)
    bq = np.asarray(bq, dtype=np.float32)
    bk = np.asarray(bk, dtype=np.float32)
    bv = np.asarray(bv, dtype=np.float32)
    gam = np.asarray(ln_gamma, dtype=np.float32)
    bet = np.asarray(ln_beta, dtype=np.float32)

    has_b = bool(np.any(bq) or np.any(bk) or np.any(bv))
    has_ln = bool(np.any(gam != 1.0) or np.any(bet))

    def q8(x, scale):
        return np.asarray(np.clip(x * scale, -240.0, 240.0), dtype=F8NP)

    hT8 = q8(np.ascontiguousarray(hs.T), SH)                 # [HS, S] fp8
    if CFG["v_hilo"]:
        hTl8 = np.asarray(np.clip(
            np.ascontiguousarray(hs.T) * SH
            - np.asarray(hT8, np.float32) * SH, -240, 240), F8NP)
    # m-major packing: block m holds WT[:, m*128:(m+1)*128] contiguously
    if CFG["qk_bf16"]:
        hTb = np.asarray(np.ascontiguousarray(hs.T), dtype=BFNP)
        wqT8 = np.ascontiguousarray(np.asarray(
            Wq.T, dtype=BFNP).reshape(HS, KC, P).transpose(1, 0, 2))
        wkT8 = np.ascontiguousarray(np.asarray(
            Wk.T, dtype=BFNP).reshape(HS, KC, P).transpose(1, 0, 2))
    else:
        wqT8 = np.ascontiguousarray(
            q8(Wq.T, SW).reshape(HS, KC, P).transpose(1, 0, 2))
        wkT8 = np.ascontiguousarray(
            q8(Wk.T, SW).reshape(HS, KC, P).transpose(1, 0, 2))
    if CFG["v_bf16"]:
        wvT8 = np.ascontiguousarray(np.asarray(Wv.T, dtype=BFNP))
    else:
        wvT8 = np.ascontiguousarray(q8(Wv.T, SW))

    tri = np.tri(P, dtype=np.float32)                        # keep k >= q
    m0x8 = np.asarray(np.tile(tri, (1, 8)), dtype=BFNP)      # [128, 1024]
    m2x8 = np.asarray(np.tile(1.0 - tri, (1, 8)), dtype=BFNP)

    # denominator-ones factor per global key: exactly 1.0 for mask==1,
    # 0.0 for mask==0 / band pads.
    vsc = 1.0 if CFG["v_bf16"] else SH * SW
    kfac = vsc * np.exp(-10000.0 * (1.0 - am)).astype(np.float32)

    in_maps = []
    for c in range(N_CORES):
        lo = c * SL - P
        hi = c * SL + SL + P
        hT_band = np.zeros((HS, SBAND), dtype=F8NP)
        km = np.zeros((SBAND,), dtype=np.float32)
        s0, s1 = max(lo, 0), min(hi, S)
        hT_band[:, s0 - lo:s1 - lo] = hT8[:, s0:s1]
        km[s0 - lo:s1 - lo] = kfac[s0:s1]
        if CFG["qk_bf16"]:
            hTb_band = np.zeros((HS, SBAND), dtype=BFNP)
            hTb_band[:, s0 - lo:s1 - lo] = hTb[:, s0:s1]
        if CFG["v_hilo"]:
            hTl_band = np.zeros((HS, SBAND), dtype=F8NP)
            hTl_band[:, s0 - lo:s1 - lo] = hTl8[:, s0:s1]
        # [P, OB*H]: per-chunk per-partition ones column, repeated per head
        kmt = np.repeat(
            km.reshape(OB, P).T.reshape(P, OB, 1), H, axis=2).reshape(P, -1)
        m = {
            "hT": hT_band,
            "wqT": wqT8, "wkT": wkT8, "wvT": wvT8,
            **({"hTb": hTb_band} if CFG["qk_bf16"] else {}),
            **({"hTl": hTl_band} if CFG["v_hilo"] else {}),
            "res": hs[c * SL:(c + 1) * SL],
            "m0x8": m0x8, "m2x8": m2x8,
            "kmask": np.asarray(kmt, dtype=BFNP),
        }
        if has_b:
            m["bq"] = bq
            m["bk"] = bk
            m["bv"] = q8(bv.reshape(1, HS), SH * SW)
        if has_ln:
            m["gam"] = np.ascontiguousarray(np.broadcast_to(gam, (P, HS)))
            m["bet"] = np.ascontiguousarray(np.broadcast_to(bet, (P, HS)))
        in_maps.append(m)
    return in_maps, has_b, has_ln


def kernel(hidden_states, attention_mask, Wq, bq, Wk, bk, Wv, bv,
           ln_gamma, ln_beta):
    in_maps, has_b, has_ln = _prep_inputs(
        hidden_states, attention_mask, Wq, bq, Wk, bk, Wv, bv,
        ln_gamma, ln_beta)
    try:
        runner = _get_runner(has_b, has_ln)
        staged = runner.stage(in_maps)
        res = runner.results(runner.run(staged))
        outs = [res[c]["out"] for c in range(N_CORES)]
    except Exception:
        # Fallback: the blessed multi-path entry point (handles both the
        # native /dev/neuron* and the axon/PJRT execution environments).
        from concourse.bass_utils import run_bass_kernel_spmd
        key = ("nc", has_b, has_ln)
        if key not in _CACHE:
            _CACHE[key] = _build(has_b, has_ln)
        r = run_bass_kernel_spmd(_CACHE[key], in_maps, list(range(N_CORES)))
        outs = [r.results[c]["out"] for c in range(N_CORES)]
    out = np.concatenate(outs, axis=0).astype(np.float32)
    return out.reshape(1, S, HS)
